# revision 1
# baseline (speedup 1.0000x reference)
"""Trainium2 Bass kernel for nn_Head (single-head causal attention, T=8).

Pure data parallel over 8 NeuronCores: per core x [4096, 8, 384] -> out
[4096, 8, 64]. The host marshals x into transposed bf16 layout
xT [ns, 3, 128c, 512tok] so the device streams contiguous tiles and never
transposes activations on-chip (PE LDWEIGHTS churn and PSUM->SBUF copy
volume were the bottleneck otherwise).

Per 512-token supertile (tokens on partitions in 4 groups of 128):
  1. DMA xT chunk [128, 3, 512] bf16 (contiguous)
  2. QK^T = [Wq|Wk]^T @ x^T -> qk [128(2h), 512] (3 MMs, W stationary)
  3. shuffle K^T (partitions 64:128) down to base 0 via SBUF->SBUF DMA
  4. V natural [tok, 64] = x @ Wv via lhsT=xT slices (12 MMs)
  5. S^T[k,q] per group: lhsT=K^T rhs=Q^T (4 MMs) -> exp (ACT) -> mask (DVE)
  6. out[q, 0:65] = S~ @ [V|1] via lhsT=S~^T (4 MMs); col 64 = softmax denom
  7. normalize: reciprocal (DVE) + per-group scaled copy (ACT)
  8. DMA out f32

bf16 on matmul paths with f32 PSUM accumulation: scale-relative error ~3e-3.
"""

import numpy as np
import ml_dtypes

import concourse.bass as bass
import concourse.mybir as mybir
from concourse import bacc
from concourse.tile import TileContext
from concourse.bass_utils import run_bass_kernel_spmd

N_CORES = 8
B_FULL = 32768
T = 8
C = 384
H = 64

BP = B_FULL // N_CORES       # batch rows per core
TOK = BP * T                 # tokens per core
ST = 512                     # tokens per supertile
G = ST // 128                # 128-token groups per supertile
NCH = C // 128               # contraction chunks
SCALE = float(C) ** -0.5

BF16 = mybir.dt.bfloat16
F32 = mybir.dt.float32
AF = mybir.ActivationFunctionType

_nc_cache = {}


def _build_nc(ns: int):
    """Build the Bass module for `ns` supertiles per core."""
    nc = bacc.Bacc("TRN2", target_bir_lowering=False, debug=False)

    xtd = nc.dram_tensor("xt", [ns, 128, NCH, ST], BF16, kind="ExternalInput")
    wqkd = nc.dram_tensor("wqk", [128, NCH, 2 * H], BF16, kind="ExternalInput")
    wvd = nc.dram_tensor("wv", [128, NCH, H], BF16, kind="ExternalInput")
    masktd = nc.dram_tensor("maskt", [128, 128], BF16, kind="ExternalInput")
    od = nc.dram_tensor("out", [ns, G, 128, H], F32, kind="ExternalOutput")

    with TileContext(nc) as tc:
        with (
            tc.tile_pool(name="const", bufs=1) as cpool,
            tc.tile_pool(name="xt", bufs=3) as xtpool,
            tc.tile_pool(name="qk", bufs=4) as qkpool,
            tc.tile_pool(name="sm", bufs=4) as smpool,
            tc.tile_pool(name="vv", bufs=4) as vpool,
            tc.tile_pool(name="oo", bufs=4) as opool,
            tc.tile_pool(name="ps_qk", bufs=2, space="PSUM") as pqk,
            tc.tile_pool(name="ps_st", bufs=2, space="PSUM") as pst,
            tc.tile_pool(name="ps_v", bufs=2, space="PSUM") as pv,
            tc.tile_pool(name="ps_o", bufs=2, space="PSUM") as po,
        ):
            maskt = cpool.tile([128, 128], BF16)
            nc.sync.dma_start(maskt, masktd[:, :])
            wqk = cpool.tile([128, NCH, 2 * H], BF16)
            nc.sync.dma_start(wqk, wqkd[:, :, :])
            wv = cpool.tile([128, NCH, H], BF16)
            nc.sync.dma_start(wv, wvd[:, :, :])

            for s in range(ns):
                # 1. load xT (already transposed + bf16 on host)
                xt_sb = xtpool.tile([128, NCH, ST], BF16, tag="xt")
                nc.sync.dma_start(xt_sb, xtd[s])

                # 2. Q^T (PE cols 0:64) and K^T (cols 64:128) col-tiled so both
                # share each rhs stream -> 2x QK throughput. K^T lands at
                # psum partitions 64:128.
                qk_ps = pqk.tile([128, ST], F32, tag="qkps")
                for j in range(NCH):
                    nc.tensor.matmul(
                        qk_ps[0:H, :],
                        lhsT=wqk[:, j, 0:H],
                        rhs=xt_sb[:, j, :],
                        start=(j == 0),
                        stop=(j == NCH - 1),
                        tile_position=(0, 0),
                    )
                    nc.tensor.matmul(
                        qk_ps[H:2 * H, :],
                        lhsT=wqk[:, j, H:2 * H],
                        rhs=xt_sb[:, j, :],
                        start=(j == 0),
                        stop=(j == NCH - 1),
                        tile_position=(0, H),
                    )
                qk_sb = qkpool.tile([128, ST], BF16, tag="qk")
                nc.vector.tensor_copy(qk_sb, qk_ps)
                qt_sb = qk_sb[0:H, :]
                # 3. shuffle K^T down to base partition 0 (S^T matmul needs
                # both operands at the same base; only DMAs cross partitions)
                kt_sb = qkpool.tile([64, ST], BF16, tag="kt")
                nc.gpsimd.dma_start(kt_sb, qk_sb[H:2 * H, :])

                # 4. V natural [tok, 64] (+ ones col)
                v_ps = pv.tile([128, G, H], F32, tag="vps")
                for g in range(G):
                    for j in range(NCH):
                        nc.tensor.matmul(
                            v_ps[:, g, :],
                            lhsT=xt_sb[:, j, g * 128:(g + 1) * 128],
                            rhs=wv[:, j, :],
                            start=(j == 0),
                            stop=(j == NCH - 1),
                        )
                v_sb = vpool.tile([128, G, H + 1], BF16, tag="v1")
                nc.scalar.copy(v_sb[:, :, 0:H], v_ps)
                nc.gpsimd.memset(v_sb[:, :, H:H + 1], 1.0)

                # 5. S^T per group, then masked exp
                st_ps = pst.tile([128, G, 128], F32, tag="stps")
                for g in range(G):
                    nc.tensor.matmul(
                        st_ps[:, g, :],
                        lhsT=kt_sb[:, g * 128:(g + 1) * 128],
                        rhs=qt_sb[:, g * 128:(g + 1) * 128],
                        start=True,
                        stop=True,
                    )
                se_sb = smpool.tile([128, G, 128], BF16, tag="se")
                nc.scalar.activation(se_sb, st_ps, AF.Exp, scale=SCALE)
                sm_sb = smpool.tile([128, G, 128], BF16, tag="sm")
                nc.vector.tensor_mul(
                    sm_sb,
                    se_sb,
                    maskt[:, None, :].to_broadcast([128, G, 128]),
                )

                # 6. out = S~ @ [V|1]
                o_ps = po.tile([128, G, H + 1], F32, tag="ops")
                for g in range(G):
                    nc.tensor.matmul(
                        o_ps[:, g, :],
                        lhsT=sm_sb[:, g, :],
                        rhs=v_sb[:, g, :],
                        start=True,
                        stop=True,
                    )

                # 7. normalize (one DVE op; recip broadcast along heads)
                recip = vpool.tile([128, G], F32, tag="recip")
                nc.vector.reciprocal(recip, o_ps[:, :, H])
                o_sb = opool.tile([128, G, H], F32, tag="o")
                nc.vector.tensor_mul(
                    o_sb,
                    o_ps[:, :, 0:H],
                    recip[:, :, None].to_broadcast([128, G, H]),
                )

                # 8. store
                nc.sync.dma_start(od[s].rearrange("g p h -> p g h"), o_sb)

    nc.finalize()
    return nc


def _consts():
    bf = ml_dtypes.bfloat16
    maskt = np.kron(
        np.eye(128 // T, dtype=np.float32),
        np.triu(np.ones((T, T), dtype=np.float32)),
    ).astype(bf)
    return maskt


def _prepare(x, Wq, Wk, Wv):
    """Returns (nc, in_maps) for the full-size problem."""
    assert x.shape == (B_FULL, T, C), x.shape
    ns = TOK // ST
    if ns not in _nc_cache:
        _nc_cache[ns] = _build_nc(ns)
    nc = _nc_cache[ns]

    bf = ml_dtypes.bfloat16
    wqk_full = np.concatenate([Wq, Wk], axis=1)  # [C, 2H]
    wqk_h = np.ascontiguousarray(
        wqk_full.reshape(NCH, 128, 2 * H).transpose(1, 0, 2)
    ).astype(bf)
    wv_h = np.ascontiguousarray(
        Wv.reshape(NCH, 128, H).transpose(1, 0, 2)
    ).astype(bf)
    maskt = _consts()

    # host-side marshalling: bf16 cast + transpose to [ns, 128c, NCH, ST]
    xb = x.reshape(N_CORES, TOK // ST, ST, NCH, 128).astype(bf)
    in_maps = []
    for c in range(N_CORES):
        xs = np.ascontiguousarray(xb[c].transpose(0, 3, 2, 1))
        in_maps.append({"xt": xs, "wqk": wqk_h, "wv": wv_h, "maskt": maskt})
    return nc, in_maps


def _gather(results):
    outs = [np.asarray(r["out"]).reshape(BP, T, H) for r in results]
    return np.concatenate(outs, axis=0)


def kernel(x, Wq, Wk, Wv):
    nc, in_maps = _prepare(x, Wq, Wk, Wv)
    res = run_bass_kernel_spmd(nc, in_maps, core_ids=list(range(N_CORES)))
    return _gather(res.results)

